# revision 3
# baseline (speedup 1.0000x reference)
"""Trainium2 Bass kernel for nn_Attention_14740327760418 (RBF-kernel attention).

Sharding: tensor-parallel over the H=8 heads, one head per NeuronCore.
Each core computes its head's full attention plus its slice of the W_o
projection; the host sums the 8 partial [B,S,D] outputs.

Math per head h (GAMMA=1, no causal mask, dropout=0):
  xn   = LayerNorm(x) * ln_w                (ln_w folded into W on host)
  Q    = xn @ Wq_h ; K = xn @ Wk_h ; V = xn @ Wv_h
  attn = exp(-(q2[s] + k2[t] - 2 qk[s,t]))  (d2 >= 28 for this data -> the
                                             reference's max(d2,0) is dead)
       = exp(-q2[s]) * exp(2 qk[s,t] - k2[t])
  out_h = (attn @ V) @ Wo_h
The exp(-k2[t]) factor rides along as a per-partition bias of the scores
exp; exp(-q2[s]) is applied as a per-partition scale on the final PSUM.

Matmuls run in float32r (full PE rate at N>=256; ~2^-13 operand rounding);
q2/k2 ones-matmuls run in exact fp32.
"""

import sys

sys.path.insert(0, "/opt/trn_rl_repo")

import numpy as np

B, S, D, H, P = 4, 1024, 256, 8, 128
DC = D // P      # 2 chunks of the embedding dim
SC = S // P      # 8 chunks of the sequence dim
NB = 512         # matmul moving-operand block
SB = S // NB     # 2 sequence blocks
LN_EPS = 1e-5

_PROGRAM_CACHE = {}


def build_program(n_iters=None):
    """Build the per-core Bass program. n_iters wraps the whole compute in a
    device-side For_i loop (for timing); None emits the plain single-shot body."""
    import concourse.bass as bass  # noqa: F401
    import concourse.mybir as mybir
    from concourse import bacc
    from concourse.tile import TileContext
    from concourse.masks import make_identity

    F32 = mybir.dt.float32
    F32R = mybir.dt.float32r
    AF = mybir.ActivationFunctionType
    ALU = mybir.AluOpType
    AX = mybir.AxisListType

    nc = bacc.Bacc(None, target_bir_lowering=False)
    x = nc.declare_dram_parameter("x", [B, S, D], F32, isOutput=False)
    wq = nc.declare_dram_parameter("wq", [D, D], F32, isOutput=False)
    wk = nc.declare_dram_parameter("wk", [D, D], F32, isOutput=False)
    wv = nc.declare_dram_parameter("wv", [D, D], F32, isOutput=False)
    wo = nc.declare_dram_parameter("wo", [D, D], F32, isOutput=False)
    out = nc.declare_dram_parameter("out", [B, S, D], F32, isOutput=True)

    with TileContext(nc) as tc:
        from contextlib import ExitStack

        with ExitStack() as ctx:
            cpool = ctx.enter_context(tc.tile_pool(name="cpool", bufs=1))
            wpool = ctx.enter_context(tc.tile_pool(name="wpool", bufs=1))
            bpool = ctx.enter_context(tc.tile_pool(name="bpool", bufs=2))
            gpool = ctx.enter_context(tc.tile_pool(name="gpool", bufs=2))
            spool = ctx.enter_context(tc.tile_pool(name="spool", bufs=3))
            ps_tr = ctx.enter_context(tc.tile_pool(name="ps_tr", bufs=2, space="PSUM"))
            ps_mm = ctx.enter_context(tc.tile_pool(name="ps_mm", bufs=4, space="PSUM"))
            ps_sm = ctx.enter_context(tc.tile_pool(name="ps_sm", bufs=2, space="PSUM"))

            def setup():
                """Constants + weights (loaded once; inside the loop for the
                timing variant so no tile crosses the For_i back-edge)."""
                ident = cpool.tile([P, P], F32, tag="ident")
                make_identity(nc, ident[:])
                ones = cpool.tile([P, 1], F32, tag="ones")
                nc.vector.memset(ones[:], 1.0)
                w_r = {}
                for name, dram in (("wq", wq), ("wk", wk), ("wv", wv), ("wo", wo)):
                    wf = spool.tile([P, DC, D], F32, tag="wload")
                    nc.sync.dma_start(wf[:],
                                      dram[:].rearrange("(dc p) e -> p dc e", p=P))
                    wr = wpool.tile([P, DC, D], F32R, tag=f"{name}_r")
                    nc.vector.tensor_copy(wr[:], wf[:])
                    w_r[name] = wr
                return ident, ones, w_r

            def batch_body(b, ident, ones, w_r):
                # --- Stage A: LayerNorm + transpose -> xnT [d, s] (fp32r) ---
                xnT = bpool.tile([P, DC, S], F32R, tag="xnT")
                for sc in range(SC):
                    xt = spool.tile([P, D], F32, tag="xt")
                    nc.sync.dma_start(xt[:], x[b, sc * P:(sc + 1) * P, :])
                    msum = spool.tile([P, 1], F32, tag="msum")
                    nc.vector.tensor_reduce(msum[:], xt[:], AX.X, ALU.add)
                    mu = spool.tile([P, 1], F32, tag="mu")
                    nc.vector.tensor_scalar_mul(mu[:], msum[:], 1.0 / D)
                    xc = spool.tile([P, D], F32, tag="xc")
                    nc.vector.tensor_scalar(xc[:], xt[:], mu[:], None, ALU.subtract)
                    sq = spool.tile([P, D], F32, tag="sq")
                    vsum = spool.tile([P, 1], F32, tag="vsum")
                    nc.scalar.activation(sq[:], xc[:], AF.Square, accum_out=vsum[:])
                    veps = spool.tile([P, 1], F32, tag="veps")
                    nc.vector.tensor_scalar(veps[:], vsum[:], 1.0 / D, LN_EPS,
                                            ALU.mult, ALU.add)
                    std = spool.tile([P, 1], F32, tag="std")
                    nc.scalar.activation(std[:], veps[:], AF.Sqrt)
                    rstd = spool.tile([P, 1], F32, tag="rstd")
                    nc.vector.reciprocal(rstd[:], std[:])
                    xn = spool.tile([P, D], F32, tag="xn")
                    nc.vector.tensor_scalar_mul(xn[:], xc[:], rstd[:])
                    for dc in range(DC):
                        pt = ps_tr.tile([P, P], F32, tag="pt")
                        nc.tensor.transpose(pt[:], xn[:, dc * P:(dc + 1) * P], ident[:])
                        nc.vector.tensor_copy(xnT[:, dc, sc * P:(sc + 1) * P], pt[:])

                # --- Stage B: projections ---
                qt = bpool.tile([P, DC, S], F32R, tag="qt")
                kt = bpool.tile([P, DC, S], F32R, tag="kt")
                vt = bpool.tile([P, SC, D], F32R, tag="vt")
                qt2 = bpool.tile([P, DC, S], F32, tag="qt2")
                kt2 = bpool.tile([P, DC, S], F32, tag="kt2")
                for dst, dst2, w in ((qt, qt2, w_r["wq"]), (kt, kt2, w_r["wk"])):
                    for eo in range(DC):
                        for sb in range(SB):
                            pp = ps_mm.tile([P, NB], F32, tag="pmm")
                            for ei in range(DC):
                                nc.tensor.matmul(
                                    pp[:], w[:, ei, eo * P:(eo + 1) * P],
                                    xnT[:, ei, sb * NB:(sb + 1) * NB],
                                    start=(ei == 0), stop=(ei == DC - 1))
                            nc.vector.tensor_copy(
                                dst[:, eo, sb * NB:(sb + 1) * NB], pp[:])
                            nc.scalar.activation(
                                dst2[:, eo, sb * NB:(sb + 1) * NB], pp[:], AF.Square)
                for t in range(SC):
                    pv = ps_mm.tile([P, NB], F32, tag="pmm")
                    for ei in range(DC):
                        nc.tensor.matmul(pv[:, :D], xnT[:, ei, t * P:(t + 1) * P],
                                         w_r["wv"][:, ei, :],
                                         start=(ei == 0), stop=(ei == DC - 1))
                    nc.vector.tensor_copy(vt[:, t, :], pv[:, :D])

                # --- Stage C: q2/k2 column vectors (exact fp32 ones-matmuls) ---
                negk2 = bpool.tile([P, SC], F32, tag="negk2")
                eq2 = bpool.tile([P, SC], F32, tag="eq2")
                for t in range(SC):
                    pk2 = ps_sm.tile([P, 1], F32, tag="psm")
                    for eo in range(DC):
                        nc.tensor.matmul(pk2[:], kt2[:, eo, t * P:(t + 1) * P],
                                         ones[:], start=(eo == 0), stop=(eo == DC - 1))
                    nc.vector.tensor_scalar_mul(negk2[:, t:t + 1], pk2[:], -1.0)
                for sc in range(SC):
                    pq2 = ps_sm.tile([P, 1], F32, tag="psm")
                    for eo in range(DC):
                        nc.tensor.matmul(pq2[:], qt2[:, eo, sc * P:(sc + 1) * P],
                                         ones[:], start=(eo == 0), stop=(eo == DC - 1))
                    nc.scalar.activation(eq2[:, sc:sc + 1], pq2[:], AF.Exp, scale=-1.0)

                # --- Stage D: scores -> exp -> attn @ V (transposed output) ---
                oT = bpool.tile([P, DC, S], F32R, tag="oT")
                for sb in range(SB):
                    gt = gpool.tile([P, SC, NB], F32R, tag="gt")
                    for t in range(SC):
                        pscr = ps_mm.tile([P, NB], F32, tag="pmm")
                        for ei in range(DC):
                            nc.tensor.matmul(pscr[:], kt[:, ei, t * P:(t + 1) * P],
                                             qt[:, ei, sb * NB:(sb + 1) * NB],
                                             start=(ei == 0), stop=(ei == DC - 1))
                        nc.scalar.activation(gt[:, t, :], pscr[:], AF.Exp,
                                             bias=negk2[:, t:t + 1], scale=2.0)
                    for ec in range(DC):
                        po = ps_mm.tile([P, NB], F32, tag="pmm")
                        for t in range(SC):
                            nc.tensor.matmul(po[:], vt[:, t, ec * P:(ec + 1) * P],
                                             gt[:, t, :],
                                             start=(t == 0), stop=(t == SC - 1))
                        nc.vector.tensor_copy(oT[:, ec, sb * NB:(sb + 1) * NB], po[:])

                # --- Stage E: W_o projection + exp(-q2[s]) scale ---
                for sc in range(SC):
                    pf = ps_mm.tile([P, NB], F32, tag="pmm")
                    for ec in range(DC):
                        nc.tensor.matmul(pf[:, :D], oT[:, ec, sc * P:(sc + 1) * P],
                                         w_r["wo"][:, ec, :],
                                         start=(ec == 0), stop=(ec == DC - 1))
                    of = spool.tile([P, D], F32, tag="of")
                    nc.vector.tensor_scalar_mul(of[:], pf[:, :D], eq2[:, sc:sc + 1])
                    nc.sync.dma_start(out[b, sc * P:(sc + 1) * P, :], of[:])

            if n_iters is None:
                ident, ones, w_r = setup()
                for b in range(B):
                    batch_body(b, ident, ones, w_r)
            else:
                with tc.For_i(0, n_iters, 1):
                    ident, ones, w_r = setup()
                    for b in range(B):
                        batch_body(b, ident, ones, w_r)

    nc.compile()
    return nc


def _get_program(n_iters=None):
    key = n_iters
    if key not in _PROGRAM_CACHE:
        _PROGRAM_CACHE[key] = build_program(n_iters)
    return _PROGRAM_CACHE[key]


def make_in_maps(x, W_q, W_k, W_v, W_o, ln_w):
    x = np.ascontiguousarray(np.asarray(x, dtype=np.float32))
    lw = np.asarray(ln_w, dtype=np.float32)[:, None]
    maps = []
    for h in range(H):
        maps.append({
            "x": x,
            "wq": np.ascontiguousarray(lw * np.asarray(W_q[h], dtype=np.float32)),
            "wk": np.ascontiguousarray(lw * np.asarray(W_k[h], dtype=np.float32)),
            "wv": np.ascontiguousarray(lw * np.asarray(W_v[h], dtype=np.float32)),
            "wo": np.ascontiguousarray(
                np.asarray(W_o[h * D:(h + 1) * D, :], dtype=np.float32)),
        })
    return maps


def kernel(x, e, p, W_q, W_k, W_v, W_o, ln_w):
    from concourse.bass_utils import run_bass_kernel_spmd

    nc = _get_program()
    in_maps = make_in_maps(x, W_q, W_k, W_v, W_o, ln_w)
    res = run_bass_kernel_spmd(nc, in_maps, list(range(H)))
    total = np.zeros((B, S, D), dtype=np.float64)
    for r in res.results:
        total += r["out"].astype(np.float64)
    return total.astype(np.float32)


# revision 8
# speedup vs baseline: 2.8601x; 2.8601x over previous
"""Trainium2 Bass kernel for nn_Attention_14740327760418 (RBF-kernel attention).

Sharding: tensor-parallel over the H=8 heads, one head per NeuronCore.
Each core computes its head's full attention plus its slice of the W_o
projection; the host sums the 8 partial [B,S,D] outputs.

Math per head h (GAMMA=1, no causal mask, dropout=0):
  xn   = LayerNorm(x) * ln_w                (ln_w folded into W on host)
  Q    = xn @ Wq_h ; K = xn @ Wk_h ; V = xn @ Wv_h
  attn = exp(-(q2[s] + k2[t] - 2 qk[s,t]))  (d2 >= 28 for this data -> the
                                             reference's max(d2,0) is dead)
       = exp(-q2[s]) * exp(2 qk[s,t] - k2[t])
  out_h = (attn @ V) @ Wo_h
The exp(-k2[t]) factor rides along as a per-partition bias of the scores
exp; exp(-q2[s]) is applied as a per-partition scale on the final PSUM.

Matmuls run in float32r (full PE rate at N>=256; ~2^-13 operand rounding);
q2/k2 ones-matmuls run in exact fp32.
"""

import sys

sys.path.insert(0, "/opt/trn_rl_repo")

import numpy as np

B, S, D, H, P = 4, 1024, 256, 8, 128
DC = D // P      # 2 chunks of the embedding dim
SC = S // P      # 8 chunks of the sequence dim
NB = 512         # matmul moving-operand block
SB = S // NB     # 2 sequence blocks
LN_EPS = 1e-5

_PROGRAM_CACHE = {}


def build_program(n_iters=None):
    """Build the per-core Bass program. n_iters wraps the whole compute in a
    device-side For_i loop (for timing); None emits the plain single-shot body."""
    import concourse.bass as bass  # noqa: F401
    import concourse.mybir as mybir
    from concourse import bacc
    from concourse.tile import TileContext
    from concourse.masks import make_identity

    F32 = mybir.dt.float32
    F32R = mybir.dt.float32r
    AF = mybir.ActivationFunctionType
    ALU = mybir.AluOpType
    AX = mybir.AxisListType

    nc = bacc.Bacc(None, target_bir_lowering=False)
    x = nc.declare_dram_parameter("x", [B, S, D], F32, isOutput=False)
    wq = nc.declare_dram_parameter("wq", [D, D], F32, isOutput=False)
    wk = nc.declare_dram_parameter("wk", [D, D], F32, isOutput=False)
    wv = nc.declare_dram_parameter("wv", [D, D], F32, isOutput=False)
    wo = nc.declare_dram_parameter("wo", [D, D], F32, isOutput=False)
    out = nc.declare_dram_parameter("out", [B, S, D], F32, isOutput=True)

    with TileContext(nc) as tc:
        from contextlib import ExitStack

        with ExitStack() as ctx:
            cpool = ctx.enter_context(tc.tile_pool(name="cpool", bufs=1))
            wpool = ctx.enter_context(tc.tile_pool(name="wpool", bufs=1))
            bpool = ctx.enter_context(tc.tile_pool(name="bpool", bufs=2))
            gpool = ctx.enter_context(tc.tile_pool(name="gpool", bufs=2))
            spool = ctx.enter_context(tc.tile_pool(name="spool", bufs=3))
            ps_tr = ctx.enter_context(tc.tile_pool(name="ps_tr", bufs=2, space="PSUM"))
            ps_mm = ctx.enter_context(tc.tile_pool(name="ps_mm", bufs=4, space="PSUM"))
            ps_sm = ctx.enter_context(tc.tile_pool(name="ps_sm", bufs=2, space="PSUM"))

            def setup():
                """Constants + weights (loaded once; inside the loop for the
                timing variant so no tile crosses the For_i back-edge)."""
                ident = cpool.tile([P, P], F32, tag="ident")
                make_identity(nc, ident[:])
                ones = cpool.tile([P, 1], F32, tag="ones")
                nc.vector.memset(ones[:], 1.0)
                w_r = {}
                for name, dram in (("wq", wq), ("wk", wk), ("wv", wv), ("wo", wo)):
                    wf = spool.tile([P, DC, D], F32, tag="wload")
                    nc.sync.dma_start(wf[:],
                                      dram[:].rearrange("(dc p) e -> p dc e", p=P))
                    wr = wpool.tile([P, DC, D], F32R, tag=f"{name}_r")
                    nc.vector.tensor_copy(wr[:], wf[:])
                    w_r[name] = wr
                return ident, ones, w_r

            def batch_body(b, ident, ones, w_r):
                # --- Stage A: LayerNorm + transpose -> xnT [d, s] (fp32r) ---
                # var = E[x^2] - mu^2; rstd = exp(-0.5*ln(var+eps)) keeps every
                # ACT func in the exp/ln/square/copy table family. All per-row
                # stats for the 8 s-chunks live in [P, SC] tiles (col = chunk),
                # so the tiny scalar chain is one instruction per step.
                xnT = bpool.tile([P, DC, S], F32R, tag="xnT")
                xts = bpool.tile([P, SC, D], F32, tag="xts")
                vsum = spool.tile([P, SC], F32, tag="vsum")
                msum = spool.tile([P, SC], F32, tag="msum")
                for sc in range(SC):
                    nc.sync.dma_start(xts[:, sc, :], x[b, sc * P:(sc + 1) * P, :])
                    sq = spool.tile([P, D], F32, tag="sq")
                    nc.scalar.activation(sq[:], xts[:, sc, :], AF.Square,
                                         accum_out=vsum[:, sc:sc + 1])
                    nc.vector.tensor_reduce(msum[:, sc:sc + 1], xts[:, sc, :],
                                            AX.X, ALU.add)
                mu = spool.tile([P, SC], F32, tag="mu")
                nc.vector.tensor_scalar_mul(mu[:], msum[:], 1.0 / D)
                mu2 = spool.tile([P, SC], F32, tag="mu2")
                nc.vector.tensor_mul(mu2[:], mu[:], mu[:])
                m2e = spool.tile([P, SC], F32, tag="m2e")
                nc.vector.tensor_scalar_add(m2e[:], mu2[:], -LN_EPS)
                veps = spool.tile([P, SC], F32, tag="veps")
                nc.vector.scalar_tensor_tensor(veps[:], vsum[:], 1.0 / D, m2e[:],
                                               ALU.mult, ALU.subtract)
                vln = spool.tile([P, SC], F32, tag="vln")
                nc.scalar.activation(vln[:], veps[:], AF.Ln)
                rstd = spool.tile([P, SC], F32, tag="rstd")
                nc.scalar.activation(rstd[:], vln[:], AF.Exp, scale=-0.5)
                musr = spool.tile([P, SC], F32, tag="musr")
                nc.vector.tensor_mul(musr[:], mu[:], rstd[:])
                for sc in range(SC):
                    xn = spool.tile([P, D], F32, tag="xn")
                    nc.vector.tensor_scalar(xn[:], xts[:, sc, :],
                                            rstd[:, sc:sc + 1], musr[:, sc:sc + 1],
                                            ALU.mult, ALU.subtract)
                    for dc in range(DC):
                        pt = ps_tr.tile([P, P], F32, tag="pt")
                        nc.tensor.transpose(pt[:], xn[:, dc * P:(dc + 1) * P], ident[:])
                        nc.vector.tensor_copy(xnT[:, dc, sc * P:(sc + 1) * P], pt[:])

                # --- Stage B: projections ---
                qt = bpool.tile([P, DC, S], F32R, tag="qt")
                kt = bpool.tile([P, DC, S], F32R, tag="kt")
                vt = bpool.tile([P, SC, D], F32R, tag="vt")
                qt2 = bpool.tile([P, DC, S], F32, tag="qt2")
                kt2 = bpool.tile([P, DC, S], F32, tag="kt2")
                for dst, dst2, w in ((qt, qt2, w_r["wq"]), (kt, kt2, w_r["wk"])):
                    for eo in range(DC):
                        for sb in range(SB):
                            pp = ps_mm.tile([P, NB], F32, tag="pmm")
                            for ei in range(DC):
                                nc.tensor.matmul(
                                    pp[:], w[:, ei, eo * P:(eo + 1) * P],
                                    xnT[:, ei, sb * NB:(sb + 1) * NB],
                                    start=(ei == 0), stop=(ei == DC - 1))
                            nc.vector.tensor_copy(
                                dst[:, eo, sb * NB:(sb + 1) * NB], pp[:])
                            nc.scalar.activation(
                                dst2[:, eo, sb * NB:(sb + 1) * NB], pp[:], AF.Square)
                for t in range(SC):
                    pv = ps_mm.tile([P, NB], F32, tag="pmm")
                    for ei in range(DC):
                        nc.tensor.matmul(pv[:, :D], xnT[:, ei, t * P:(t + 1) * P],
                                         w_r["wv"][:, ei, :],
                                         start=(ei == 0), stop=(ei == DC - 1))
                    nc.vector.tensor_copy(vt[:, t, :], pv[:, :D])

                # --- Stage C: q2/k2 column vectors (exact fp32 ones-matmuls) ---
                # Pre-sum the two embedding chunks so each column needs 1 matmul.
                kt2s = bpool.tile([P, S], F32, tag="kt2s")
                nc.gpsimd.tensor_add(kt2s[:], kt2[:, 0, :], kt2[:, 1, :])
                qt2s = bpool.tile([P, S], F32, tag="qt2s")
                nc.gpsimd.tensor_add(qt2s[:], qt2[:, 0, :], qt2[:, 1, :])
                negk2 = bpool.tile([P, SC], F32, tag="negk2")
                eq2 = bpool.tile([P, SC], F32, tag="eq2")
                for t in range(SC):
                    pk2 = ps_sm.tile([P, 1], F32, tag="psm")
                    nc.tensor.matmul(pk2[:], kt2s[:, t * P:(t + 1) * P],
                                     ones[:], start=True, stop=True)
                    nc.vector.tensor_scalar_mul(negk2[:, t:t + 1], pk2[:], -1.0)
                for sc in range(SC):
                    pq2 = ps_sm.tile([P, 1], F32, tag="psm")
                    nc.tensor.matmul(pq2[:], qt2s[:, sc * P:(sc + 1) * P],
                                     ones[:], start=True, stop=True)
                    nc.scalar.activation(eq2[:, sc:sc + 1], pq2[:], AF.Exp, scale=-1.0)

                # --- Stage D: scores -> exp -> attn @ V (transposed output) ---
                oT = bpool.tile([P, DC, S], F32R, tag="oT")
                for sb in range(SB):
                    gt = gpool.tile([P, SC, NB], F32R, tag="gt")
                    for t in range(SC):
                        pscr = ps_mm.tile([P, NB], F32, tag="pmm")
                        for ei in range(DC):
                            nc.tensor.matmul(pscr[:], kt[:, ei, t * P:(t + 1) * P],
                                             qt[:, ei, sb * NB:(sb + 1) * NB],
                                             start=(ei == 0), stop=(ei == DC - 1))
                        nc.scalar.activation(gt[:, t, :], pscr[:], AF.Exp,
                                             bias=negk2[:, t:t + 1], scale=2.0)
                    for ec in range(DC):
                        po = ps_mm.tile([P, NB], F32, tag="pmm")
                        for t in range(SC):
                            nc.tensor.matmul(po[:], vt[:, t, ec * P:(ec + 1) * P],
                                             gt[:, t, :],
                                             start=(t == 0), stop=(t == SC - 1))
                        nc.vector.tensor_copy(oT[:, ec, sb * NB:(sb + 1) * NB], po[:])

                # --- Stage E: W_o projection + exp(-q2[s]) scale ---
                for sc in range(SC):
                    pf = ps_mm.tile([P, NB], F32, tag="pmm")
                    for ec in range(DC):
                        nc.tensor.matmul(pf[:, :D], oT[:, ec, sc * P:(sc + 1) * P],
                                         w_r["wo"][:, ec, :],
                                         start=(ec == 0), stop=(ec == DC - 1))
                    of = spool.tile([P, D], F32, tag="of")
                    nc.vector.tensor_scalar_mul(of[:], pf[:, :D], eq2[:, sc:sc + 1])
                    nc.sync.dma_start(out[b, sc * P:(sc + 1) * P, :], of[:])

            if n_iters is None:
                ident, ones, w_r = setup()
                for b in range(B):
                    batch_body(b, ident, ones, w_r)
            else:
                with tc.For_i(0, n_iters, 1):
                    ident, ones, w_r = setup()
                    for b in range(B):
                        batch_body(b, ident, ones, w_r)

    nc.compile()
    return nc


def _get_program(n_iters=None):
    key = n_iters
    if key not in _PROGRAM_CACHE:
        _PROGRAM_CACHE[key] = build_program(n_iters)
    return _PROGRAM_CACHE[key]


def make_in_maps(x, W_q, W_k, W_v, W_o, ln_w):
    x = np.ascontiguousarray(np.asarray(x, dtype=np.float32))
    lw = np.asarray(ln_w, dtype=np.float32)[:, None]
    maps = []
    for h in range(H):
        maps.append({
            "x": x,
            "wq": np.ascontiguousarray(lw * np.asarray(W_q[h], dtype=np.float32)),
            "wk": np.ascontiguousarray(lw * np.asarray(W_k[h], dtype=np.float32)),
            "wv": np.ascontiguousarray(lw * np.asarray(W_v[h], dtype=np.float32)),
            "wo": np.ascontiguousarray(
                np.asarray(W_o[h * D:(h + 1) * D, :], dtype=np.float32)),
        })
    return maps


def kernel(x, e, p, W_q, W_k, W_v, W_o, ln_w):
    from concourse.bass_utils import run_bass_kernel_spmd

    nc = _get_program()
    in_maps = make_in_maps(x, W_q, W_k, W_v, W_o, ln_w)
    res = run_bass_kernel_spmd(nc, in_maps, list(range(H)))
    total = np.zeros((B, S, D), dtype=np.float64)
    for r in res.results:
        total += r["out"].astype(np.float64)
    return total.astype(np.float32)
